# revision 15
# baseline (speedup 1.0000x reference)
"""Trainium2 Bass kernel for one burst-mode CIF neuron step.

Reference math (closed form of the two burst while-loops):
    m      = mem + x
    k_pos  = max(ceil((m - th)/th), 0)            # positive burst count
    m1     = m - k_pos*th
    scu    = round((spike_count + k_pos*th)/th)
    j_mem  = max(ceil((-m1 - th)/th), 0)
    k_neg  = min(j_mem, max(scu, 0))
    spike  = (k_pos - k_neg)*th

On-device reformulation: the two loops are mutually exclusive (if k_pos > 0
then m1 in (0, th], so j_mem = 0).  With q = m/th:
    k_pos = relu(ceil(q) - 1)
    k_neg = min(relu(-floor(q) - 1), spike_count/th)
    spike = (k_pos - k_neg)*th
(spike_count/th is a non-negative near-integer; using it unrounded inside
the min only perturbs the result by ~1 ulp when it wins the min.)

ceil/floor come from the fp32 round-to-nearest magic constant C = 1.5*2^23:
for |v| < 2^22, rint(v) == (v + C) - C.  Let ts2 = (q + 0.5) + C
= C + ceil(q) (exact except q an exact odd integer, measure-zero on this
data and equally boundary-sensitive in the reference):
    k_pos = relu(ts2 - (C+1))
    j_mem = relu(-floor(q) - 1) = relu(-(ceil(q)-1) - 1) = relu(-ts2 + C)
Measured end-to-end L2 relative error vs the jax reference is ~2e-4
(a handful of 67M elements sit on ceil boundaries and flip by one th).

Sharding: pure elementwise -> flatten [B,T,H] to [B*T, H] rows and shard
rows across the 8 cores data-parallel; threshold [H] replicated.  x and
mem are packed host-side into one [rows, 2H] array so each tile arrives
in ONE DMA (the hardware allows only one semaphore wait per instruction,
so a 2-input op may depend on at most one fresh DMA).

Engine split per [128, 2048] half-tile (DVE is the 1-elem/cycle
bottleneck engine; DMA ~358 GB/s is the target roofline):
    DVE : m = x+mem, ts2 (tensor_scalar 2x), kn = min(jm,s), d = kp-kn,
          out = d*TH
    ACT : kp = Relu(ts2 - (C+1)), jm = Relu(-ts2 + C)
    Pool: q = m*R, s = sc*R  (+ SWDGE descriptor gen for sc loads)
The hardware allows one semaphore wait per instruction (Bacc's
generate_event_semaphores splits the rest, but each split costs an extra
instruction), so the dataflow is arranged so nearly every op has at most
one unobserved cross-engine dependency, with tiny observer copies
pre-observing the others.
"""

import numpy as np

B, T, H = 4, 4096, 4096
N_CORES = 8
ROWS_PER_CORE = (B * T) // N_CORES  # 2048
P = 128
HALF = 2048
MAGIC = 12582912.0  # 1.5 * 2^23

_NC_CACHE: dict = {}


def build_nc(rows: int = ROWS_PER_CORE):
    """Build the per-core Bass program (identical on all cores)."""
    from contextlib import ExitStack

    import concourse.bacc as bacc
    import concourse.bass as bass
    import concourse.mybir as mybir
    from bass_rust import add_dep_helper
    from concourse.tile import TileContext

    f32 = mybir.dt.float32
    Alu = mybir.AluOpType
    Act = mybir.ActivationFunctionType

    assert rows % P == 0
    n_blocks = rows // P

    nc = bacc.Bacc("TRN2", target_bir_lowering=False, debug=False)
    xm_d = nc.dram_tensor("xm", [rows, 2 * H], f32, kind="ExternalInput").ap()
    s_d = nc.dram_tensor("spike_count", [rows, H], f32, kind="ExternalInput").ap()
    t_d = nc.dram_tensor("threshold", [H], f32, kind="ExternalInput").ap()
    o_d = nc.dram_tensor("spike", [rows, H], f32, kind="ExternalOutput").ap()
    r_d = nc.dram_tensor("recip_scratch", [H], f32, kind="Internal").ap()

    with TileContext(nc) as tc, ExitStack() as ctx:
        consts = ctx.enter_context(tc.tile_pool(name="consts", bufs=1))
        io = ctx.enter_context(tc.tile_pool(name="io", bufs=2))
        work = ctx.enter_context(tc.tile_pool(name="work", bufs=1))
        once = ctx.enter_context(tc.tile_pool(name="once", bufs=1))

        # ---- one-time threshold setup ----
        # th broadcast to all 128 partitions via a step-0 partition DMA.
        TH = consts.tile([P, H], f32, tag="TH")
        th_bcast = bass.AP(
            tensor=t_d.tensor, offset=t_d.offset, ap=[[0, P]] + list(t_d.ap)
        )
        nc.gpsimd.dma_start(out=TH[:], in_=th_bcast)

        # reciprocal computed on a [128, H/128] relayout (H distinct values,
        # not 128*H), bounced through DRAM, then broadcast like th.
        th_pn = consts.tile([P, H // P], f32, tag="th_pn")
        nc.sync.dma_start(out=th_pn[:], in_=t_d.rearrange("(n p) -> p n", p=P))
        r_pn = consts.tile([P, H // P], f32, tag="r_pn")
        nc.vector.reciprocal(r_pn[:], th_pn[:])
        nc.sync.dma_start(out=r_d.rearrange("(n p) -> p n", p=P), in_=r_pn[:])
        R = consts.tile([P, H], f32, tag="R")
        r_bcast = bass.AP(
            tensor=r_d.tensor, offset=r_d.offset, ap=[[0, P]] + list(r_d.ap)
        )
        nc.gpsimd.dma_start(out=R[:], in_=r_bcast)

        # per-partition bias vectors for the Relu activations
        bias_kp = consts.tile([P, 1], f32, tag="bias_kp")
        nc.vector.memset(bias_kp[:], -(MAGIC + 1.0))
        bias_jm = consts.tile([P, 1], f32, tag="bias_jm")
        nc.vector.memset(bias_jm[:], MAGIC)

        # Pool pre-touches R so its later ops never wait on R's DMA.
        pool_dummy = consts.tile([P, 1], f32, tag="pool_dummy")
        nc.gpsimd.tensor_copy(pool_dummy[:], R[:, 0:1])

        dve_dummy = consts.tile([P, 1], f32, tag="dve_dummy")

        # ---- main loop: n_blocks row-blocks x 2 column halves ----
        # xm DRAM view: row = nb*128 + p; col = t*H + hh*HALF + c
        xm_t = xm_d.rearrange(
            "(nb p) (t hh c) -> nb hh p t c", p=P, t=2, hh=2, c=HALF
        )
        s_t = s_d.rearrange("(nb p) h -> nb p h", p=P)
        o_t = o_d.rearrange("(nb p) h -> nb p h", p=P)

        for b in range(n_blocks):
            # spike_count for the whole block, via SWDGE (Pool-issued) so
            # the slot-reuse WAR against Pool's own accesses is free.
            tsc = io.tile([P, H], f32, tag="s")
            nc.gpsimd.dma_start(out=tsc[:], in_=s_t[b])
            tout = io.tile([P, H], f32, tag="out")

            for hf in range(2):
                sl = bass.ts(hf, HALF)
                txm = io.tile([P, 2, HALF], f32, tag="xm")
                nc.sync.dma_start(out=txm[:], in_=xm_t[b, hf])

                # m = x + mem   (DVE; only cross dep = the xm DMA)
                tm = work.tile([P, HALF], f32, tag="tm")
                nc.vector.tensor_tensor(tm[:], txm[:, 0, :], txm[:, 1, :], Alu.add)
                # q = m * (1/th)   (Pool)
                tq = work.tile([P, HALF], f32, tag="tq")
                nc.gpsimd.tensor_tensor(tq[:], tm[:], R[:, sl], Alu.mult)
                # ts2 = (q + 0.5) + C = C + ceil(q)   (DVE 2x tensor_scalar)
                tkp = work.tile([P, HALF], f32, tag="tkp")
                nc.vector.tensor_scalar(tkp[:], tq[:], 0.5, MAGIC, Alu.add, Alu.add)
                # k_pos = relu(ts2 - (C+1)), then j_mem = relu(-ts2 + C); kp
                # first so the min's wait on jm's tick also covers kp.
                tkq = work.tile([P, HALF], f32, tag="tkq")
                nc.scalar.activation(tkq[:], tkp[:], Act.Relu, bias=bias_kp[:])
                tj = work.tile([P, HALF], f32, tag="tj")
                nc.scalar.activation(
                    tj[:], tkp[:], Act.Relu, bias=bias_jm[:], scale=-1.0
                )
                # s = spike_count / th   (Pool)
                nc.gpsimd.tensor_tensor(tsc[:, sl], tsc[:, sl], R[:, sl], Alu.mult)
                # DVE pre-observes Pool's s tick, then the min needs only
                # the ACT wait.
                c0 = hf * HALF
                i_obs = nc.vector.tensor_copy(dve_dummy[:], tsc[:, c0 : c0 + 1])
                # kn = min(j_mem, s)
                i_min = nc.vector.tensor_tensor(tj[:], tj[:], tsc[:, sl], Alu.min)
                add_dep_helper(i_min.ins, i_obs.ins, sync=False, reason="obs<min")
                # d = k_pos - kn   (ACT tick already observed via the min)
                nc.vector.tensor_tensor(tkq[:], tkq[:], tj[:], Alu.subtract)
                # spike = d * th
                nc.vector.tensor_tensor(tout[:, sl], tkq[:], TH[:, sl], Alu.mult)

            nc.sync.dma_start(out=o_t[b], in_=tout[:])

    return nc


def kernel(**inputs: np.ndarray) -> np.ndarray:
    from concourse.bass_utils import run_bass_kernel_spmd

    x = np.ascontiguousarray(inputs["x"], dtype=np.float32).reshape(B * T, H)
    mem = np.ascontiguousarray(inputs["mem"], dtype=np.float32).reshape(B * T, H)
    sc = np.ascontiguousarray(inputs["spike_count"], dtype=np.float32).reshape(
        B * T, H
    )
    th = np.ascontiguousarray(inputs["threshold"], dtype=np.float32)
    xm = np.concatenate([x, mem], axis=1)  # [B*T, 2H]

    if "nc" not in _NC_CACHE:
        nc = build_nc()
        nc.finalize()
        _NC_CACHE["nc"] = nc
    nc = _NC_CACHE["nc"]

    r = ROWS_PER_CORE
    in_maps = [
        {
            "xm": xm[c * r : (c + 1) * r],
            "spike_count": sc[c * r : (c + 1) * r],
            "threshold": th,
        }
        for c in range(N_CORES)
    ]
    res = run_bass_kernel_spmd(nc, in_maps, core_ids=list(range(N_CORES)))
    out = np.concatenate([res.results[c]["spike"] for c in range(N_CORES)], axis=0)
    return out.reshape(B, T, H).astype(np.float32, copy=False)


# revision 20
# speedup vs baseline: 1.0639x; 1.0639x over previous
"""Trainium2 Bass kernel for one burst-mode CIF neuron step.

Reference math (closed form of the two burst while-loops):
    m      = mem + x
    k_pos  = max(ceil((m - th)/th), 0)            # positive burst count
    m1     = m - k_pos*th
    scu    = round((spike_count + k_pos*th)/th)
    j_mem  = max(ceil((-m1 - th)/th), 0)
    k_neg  = min(j_mem, max(scu, 0))
    spike  = (k_pos - k_neg)*th

On-device reformulation: the two loops are mutually exclusive (if k_pos > 0
then m1 in (0, th], so j_mem = 0).  With q = m/th:
    k_pos = relu(ceil(q) - 1)
    k_neg = min(relu(-floor(q) - 1), spike_count/th)
    spike = (k_pos - k_neg)*th
(spike_count/th is a non-negative near-integer; using it unrounded inside
the min only perturbs the result by ~1 ulp when it wins the min.)

ceil/floor come from the fp32 round-to-nearest magic constant C = 1.5*2^23:
for |v| < 2^22, rint(v) == (v + C) - C.  Let ts2 = (q + 0.5) + C
= C + ceil(q) (exact except q an exact odd integer, measure-zero on this
data and equally boundary-sensitive in the reference):
    k_pos = relu(ts2 - (C+1))
    j_mem = relu(-floor(q) - 1) = relu(-(ceil(q)-1) - 1) = relu(-ts2 + C)
Measured end-to-end L2 relative error vs the jax reference is ~2e-4
(a handful of 67M elements sit on ceil boundaries and flip by one th).

Sharding: pure elementwise -> flatten [B,T,H] to [B*T, H] rows and shard
rows across the 8 cores data-parallel; threshold [H] replicated.  x and
mem are packed host-side into one [rows, 2H] array so each tile arrives
in ONE DMA (the hardware allows only one semaphore wait per instruction,
so a 2-input op may depend on at most one fresh DMA).

Engine split per [128, 2048] half-tile (DVE is the 1-elem/cycle
bottleneck engine; DMA ~358 GB/s is the target roofline):
    DVE : m = x+mem, ts2 (tensor_scalar 2x), kn = min(jm,s), d = kp-kn,
          out = d*TH
    ACT : kp = Relu(ts2 - (C+1)), jm = Relu(-ts2 + C)
    Pool: q = m*R, s = sc*R  (+ SWDGE descriptor gen for sc loads)
The hardware allows one semaphore wait per instruction (Bacc's
generate_event_semaphores splits the rest, but each split costs an extra
instruction), so the dataflow is arranged so nearly every op has at most
one unobserved cross-engine dependency, with tiny observer copies
pre-observing the others.
"""

import numpy as np

B, T, H = 4, 4096, 4096
N_CORES = 8
ROWS_PER_CORE = (B * T) // N_CORES  # 2048
P = 128
HALF = 2048
MAGIC = 12582912.0  # 1.5 * 2^23

_NC_CACHE: dict = {}


def build_nc(rows: int = ROWS_PER_CORE):
    """Build the per-core Bass program (identical on all cores)."""
    from contextlib import ExitStack

    import concourse.bacc as bacc
    import concourse.bass as bass
    import concourse.mybir as mybir
    from bass_rust import add_dep_helper
    from concourse.tile import TileContext

    f32 = mybir.dt.float32
    Alu = mybir.AluOpType
    Act = mybir.ActivationFunctionType

    assert rows % P == 0
    n_blocks = rows // P

    nc = bacc.Bacc("TRN2", target_bir_lowering=False, debug=False)
    xm_d = nc.dram_tensor("xm", [rows, 2 * H], f32, kind="ExternalInput").ap()
    s_d = nc.dram_tensor("spike_count", [rows, H], f32, kind="ExternalInput").ap()
    t_d = nc.dram_tensor("threshold", [H], f32, kind="ExternalInput").ap()
    o_d = nc.dram_tensor("spike", [rows, H], f32, kind="ExternalOutput").ap()
    r_d = nc.dram_tensor("recip_scratch", [H], f32, kind="Internal").ap()

    with TileContext(nc) as tc, ExitStack() as ctx:
        consts = ctx.enter_context(tc.tile_pool(name="consts", bufs=1))
        io = ctx.enter_context(tc.tile_pool(name="io", bufs=2))
        work = ctx.enter_context(tc.tile_pool(name="work", bufs=1))
        work2 = ctx.enter_context(tc.tile_pool(name="work2", bufs=2))
        once = ctx.enter_context(tc.tile_pool(name="once", bufs=1))

        # ---- one-time threshold setup ----
        # th broadcast to all 128 partitions via a step-0 partition DMA.
        TH = consts.tile([P, H], f32, tag="TH")
        th_bcast = bass.AP(
            tensor=t_d.tensor, offset=t_d.offset, ap=[[0, P]] + list(t_d.ap)
        )
        nc.gpsimd.dma_start(out=TH[:], in_=th_bcast)

        # reciprocal computed on a [128, H/128] relayout (H distinct values,
        # not 128*H), bounced through DRAM, then broadcast like th.
        th_pn = consts.tile([P, H // P], f32, tag="th_pn")
        nc.sync.dma_start(out=th_pn[:], in_=t_d.rearrange("(n p) -> p n", p=P))
        r_pn = consts.tile([P, H // P], f32, tag="r_pn")
        nc.vector.reciprocal(r_pn[:], th_pn[:])
        nc.sync.dma_start(out=r_d.rearrange("(n p) -> p n", p=P), in_=r_pn[:])
        R = consts.tile([P, H], f32, tag="R")
        r_bcast = bass.AP(
            tensor=r_d.tensor, offset=r_d.offset, ap=[[0, P]] + list(r_d.ap)
        )
        nc.gpsimd.dma_start(out=R[:], in_=r_bcast)

        # per-partition bias vectors for the ACT ops
        bias_kp = consts.tile([P, 1], f32, tag="bias_kp")
        nc.vector.memset(bias_kp[:], -(MAGIC + 1.0))
        bias_jm = consts.tile([P, 1], f32, tag="bias_jm")
        nc.vector.memset(bias_jm[:], MAGIC)
        bias_half = consts.tile([P, 1], f32, tag="bias_half")
        nc.vector.memset(bias_half[:], 0.5)
        bias_C = consts.tile([P, 1], f32, tag="bias_C")
        nc.vector.memset(bias_C[:], MAGIC)

        # Pool pre-touches R so its later ops never wait on R's DMA.
        pool_dummy = consts.tile([P, 1], f32, tag="pool_dummy")
        nc.gpsimd.tensor_copy(pool_dummy[:], R[:, 0:1])

        dve_dummy = consts.tile([P, 1], f32, tag="dve_dummy")

        # ---- main loop: n_blocks row-blocks x 2 column halves ----
        # xm DRAM view: row = nb*128 + p; col = t*H + hh*HALF + c
        xm_t = xm_d.rearrange(
            "(nb p) (t hh c) -> nb hh p t c", p=P, t=2, hh=2, c=HALF
        )
        s_t = s_d.rearrange("(nb p) h -> nb p h", p=P)
        o_t = o_d.rearrange("(nb p) h -> nb p h", p=P)

        for b in range(n_blocks):
            # spike_count for the whole block, via SWDGE (Pool-issued) so
            # the slot-reuse WAR against Pool's own accesses is free.
            tsc = io.tile([P, H], f32, tag="s")
            nc.gpsimd.dma_start(out=tsc[:], in_=s_t[b])
            tout = io.tile([P, H], f32, tag="out")

            for hf in range(2):
                sl = bass.ts(hf, HALF)
                txm = io.tile([P, 2, HALF], f32, tag="xm")
                nc.sync.dma_start(out=txm[:], in_=xm_t[b, hf])

                # m = x + mem   (DVE; only cross dep = the xm DMA)
                tm = work.tile([P, HALF], f32, tag="tm")
                nc.vector.tensor_tensor(tm[:], txm[:, 0, :], txm[:, 1, :], Alu.add)
                # q = m * (1/th), in place; alternate Pool/DVE to balance
                # (Pool work also steals a shared SBUF port from DVE).
                if hf == 0:
                    nc.gpsimd.tensor_tensor(tm[:], tm[:], R[:, sl], Alu.mult)
                else:
                    nc.vector.tensor_tensor(tm[:], tm[:], R[:, sl], Alu.mult)
                # rounding chain on ACT (own SBUF port, plenty of headroom):
                # ta = q + 0.5 ; ts2 = ta + C = C + ceil(q)
                ta = work2.tile([P, HALF], f32, tag="tab")
                nc.scalar.activation(ta[:], tm[:], Act.Identity, bias=bias_half[:])
                tb = work2.tile([P, HALF], f32, tag="tab")
                nc.scalar.activation(tb[:], ta[:], Act.Identity, bias=bias_C[:])
                # k_pos = relu(ts2 - (C+1)), then j_mem = relu(-ts2 + C); kp
                # first so the min's wait on jm's tick also covers kp.
                tkq = work2.tile([P, HALF], f32, tag="tkq")
                nc.scalar.activation(tkq[:], tb[:], Act.Relu, bias=bias_kp[:])
                tj = work2.tile([P, HALF], f32, tag="tj")
                nc.scalar.activation(
                    tj[:], tb[:], Act.Relu, bias=bias_jm[:], scale=-1.0
                )
                # s = spike_count / th   (Pool, in place)
                nc.gpsimd.tensor_tensor(tsc[:, sl], tsc[:, sl], R[:, sl], Alu.mult)
                # DVE pre-observes Pool's s tick, then the min needs only
                # the ACT wait.
                c0 = hf * HALF
                i_obs = nc.vector.tensor_copy(dve_dummy[:], tsc[:, c0 : c0 + 1])
                # kn = min(j_mem, s)
                i_min = nc.vector.tensor_tensor(tj[:], tj[:], tsc[:, sl], Alu.min)
                add_dep_helper(i_min.ins, i_obs.ins, sync=False, reason="obs<min")
                # d = k_pos - kn   (ACT tick already observed via the min)
                nc.vector.tensor_tensor(tkq[:], tkq[:], tj[:], Alu.subtract)
                # spike = d * th
                nc.vector.tensor_tensor(tout[:, sl], tkq[:], TH[:, sl], Alu.mult)

            nc.sync.dma_start(out=o_t[b], in_=tout[:])

    return nc


def kernel(**inputs: np.ndarray) -> np.ndarray:
    from concourse.bass_utils import run_bass_kernel_spmd

    x = np.ascontiguousarray(inputs["x"], dtype=np.float32).reshape(B * T, H)
    mem = np.ascontiguousarray(inputs["mem"], dtype=np.float32).reshape(B * T, H)
    sc = np.ascontiguousarray(inputs["spike_count"], dtype=np.float32).reshape(
        B * T, H
    )
    th = np.ascontiguousarray(inputs["threshold"], dtype=np.float32)
    xm = np.concatenate([x, mem], axis=1)  # [B*T, 2H]

    if "nc" not in _NC_CACHE:
        nc = build_nc()
        nc.finalize()
        _NC_CACHE["nc"] = nc
    nc = _NC_CACHE["nc"]

    r = ROWS_PER_CORE
    in_maps = [
        {
            "xm": xm[c * r : (c + 1) * r],
            "spike_count": sc[c * r : (c + 1) * r],
            "threshold": th,
        }
        for c in range(N_CORES)
    ]
    res = run_bass_kernel_spmd(nc, in_maps, core_ids=list(range(N_CORES)))
    out = np.concatenate([res.results[c]["spike"] for c in range(N_CORES)], axis=0)
    return out.reshape(B, T, H).astype(np.float32, copy=False)


# revision 26
# speedup vs baseline: 1.1948x; 1.1231x over previous
"""Trainium2 Bass kernel for one burst-mode CIF neuron step.

Reference math (closed form of the two burst while-loops):
    m      = mem + x
    k_pos  = max(ceil((m - th)/th), 0)            # positive burst count
    m1     = m - k_pos*th
    scu    = round((spike_count + k_pos*th)/th)
    j_mem  = max(ceil((-m1 - th)/th), 0)
    k_neg  = min(j_mem, max(scu, 0))
    spike  = (k_pos - k_neg)*th

On-device reformulation: the two loops are mutually exclusive (if k_pos > 0
then m1 in (0, th], so j_mem = 0).  With q = m/th:
    k_pos = relu(ceil(q) - 1)
    k_neg = min(relu(-floor(q) - 1), spike_count/th)
    spike = (k_pos - k_neg)*th
(spike_count/th is a non-negative near-integer; using it unrounded inside
the min only perturbs the result by ~1 ulp when it wins the min.)

ceil/floor come from the fp32 round-to-nearest magic constant C = 1.5*2^23:
for |v| < 2^22, rint(v) == (v + C) - C.  Let ts2 = (q + 0.5) + C
= C + ceil(q) (exact except q an exact odd integer, measure-zero on this
data and equally boundary-sensitive in the reference):
    k_pos = relu(ts2 - (C+1))
    j_mem = relu(-floor(q) - 1) = relu(-(ceil(q)-1) - 1) = relu(-ts2 + C)
Measured end-to-end L2 relative error vs the jax reference is ~2e-4
(a handful of 67M elements sit on ceil boundaries and flip by one th).

Sharding: pure elementwise -> flatten [B,T,H] to [B*T, H] rows and shard
rows across the 8 cores data-parallel; threshold [H] replicated.  x and
mem are packed host-side into one [rows, 2H] array so each tile arrives
in ONE DMA (the hardware allows only one semaphore wait per instruction,
so a 2-input op may depend on at most one fresh DMA).

Engine split per [128, 2048] half-tile (DVE is the 1-elem/cycle
bottleneck engine; DMA ~358 GB/s is the target roofline):
    DVE : m = x+mem, ts2 (tensor_scalar 2x), kn = min(jm,s), d = kp-kn,
          out = d*TH
    ACT : kp = Relu(ts2 - (C+1)), jm = Relu(-ts2 + C)
    Pool: q = m*R, s = sc*R  (+ SWDGE descriptor gen for sc loads)
The hardware allows one semaphore wait per instruction (Bacc's
generate_event_semaphores splits the rest, but each split costs an extra
instruction), so the dataflow is arranged so nearly every op has at most
one unobserved cross-engine dependency, with tiny observer copies
pre-observing the others.
"""

import numpy as np

B, T, H = 4, 4096, 4096
N_CORES = 8
ROWS_PER_CORE = (B * T) // N_CORES  # 2048
P = 128
HALF = 2048
MAGIC = 12582912.0  # 1.5 * 2^23

_NC_CACHE: dict = {}


def build_nc(rows: int = ROWS_PER_CORE):
    """Build the per-core Bass program (identical on all cores)."""
    from contextlib import ExitStack

    import concourse.bacc as bacc
    import concourse.bass as bass
    import concourse.mybir as mybir
    from bass_rust import add_dep_helper
    from concourse.tile import TileContext

    f32 = mybir.dt.float32
    Alu = mybir.AluOpType
    Act = mybir.ActivationFunctionType

    assert rows % P == 0
    n_blocks = rows // P

    nc = bacc.Bacc("TRN2", target_bir_lowering=False, debug=False)
    xm_d = nc.dram_tensor("xm", [rows, 2 * H], f32, kind="ExternalInput").ap()
    s_d = nc.dram_tensor("spike_count", [rows, H], f32, kind="ExternalInput").ap()
    t_d = nc.dram_tensor("threshold", [H], f32, kind="ExternalInput").ap()
    e_d = nc.dram_tensor("eye", [P, 2 * P], f32, kind="ExternalInput").ap()
    o_d = nc.dram_tensor("spike", [rows, H], f32, kind="ExternalOutput").ap()
    r_d = nc.dram_tensor("recip_scratch", [H], f32, kind="Internal").ap()

    with TileContext(nc) as tc, ExitStack() as ctx:
        consts = ctx.enter_context(tc.tile_pool(name="consts", bufs=1))
        io = ctx.enter_context(tc.tile_pool(name="io", bufs=2))
        work = ctx.enter_context(tc.tile_pool(name="work", bufs=1))
        work2 = ctx.enter_context(tc.tile_pool(name="work2", bufs=2))
        once = ctx.enter_context(tc.tile_pool(name="once", bufs=1))

        # ---- one-time threshold setup ----
        # th broadcast to all 128 partitions via a step-0 partition DMA.
        TH = consts.tile([P, H], f32, tag="TH")
        th_bcast = bass.AP(
            tensor=t_d.tensor, offset=t_d.offset, ap=[[0, P]] + list(t_d.ap)
        )
        nc.gpsimd.dma_start(out=TH[:], in_=th_bcast)

        # reciprocal computed on a [128, H/128] relayout (H distinct values,
        # not 128*H), bounced through DRAM, then broadcast like th.
        th_pn = consts.tile([P, H // P], f32, tag="th_pn")
        nc.sync.dma_start(out=th_pn[:], in_=t_d.rearrange("(n p) -> p n", p=P))
        r_pn = consts.tile([P, H // P], f32, tag="r_pn")
        nc.vector.reciprocal(r_pn[:], th_pn[:])
        nc.sync.dma_start(out=r_d.rearrange("(n p) -> p n", p=P), in_=r_pn[:])
        R = consts.tile([P, H], f32, tag="R")
        r_bcast = bass.AP(
            tensor=r_d.tensor, offset=r_d.offset, ap=[[0, P]] + list(r_d.ap)
        )
        nc.gpsimd.dma_start(out=R[:], in_=r_bcast)

        # per-partition bias vectors for the ACT ops
        bias_kp = consts.tile([P, 1], f32, tag="bias_kp")
        nc.vector.memset(bias_kp[:], -(MAGIC + 1.0))
        bias_jm = consts.tile([P, 1], f32, tag="bias_jm")
        nc.vector.memset(bias_jm[:], MAGIC)
        bias_half = consts.tile([P, 1], f32, tag="bias_half")
        nc.vector.memset(bias_half[:], 0.5)
        bias_C = consts.tile([P, 1], f32, tag="bias_C")
        nc.vector.memset(bias_C[:], MAGIC)

        # Pool pre-touches R so its later ops never wait on R's DMA.
        pool_dummy = consts.tile([P, 1], f32, tag="pool_dummy")
        nc.gpsimd.tensor_copy(pool_dummy[:], R[:, 0:1])

        dve_dummy = consts.tile([P, 1], f32, tag="dve_dummy")

        # [I | -I] for the PE-side d = kp - kn (exact: kp/kn are small ints)
        eye = consts.tile([P, 2 * P], f32, tag="eye")
        nc.sync.dma_start(out=eye[:], in_=e_d)
        psum = ctx.enter_context(tc.tile_pool(name="psum", bufs=2, space="PSUM"))
        NMM = 512  # matmul free-dim cap (one PSUM bank)

        # ---- main loop: n_blocks row-blocks x 2 column halves ----
        # xm DRAM view: row = nb*128 + p; col = t*H + hh*HALF + c
        xm_t = xm_d.rearrange(
            "(nb p) (t hh c) -> nb hh p t c", p=P, t=2, hh=2, c=HALF
        )
        s_t = s_d.rearrange("(nb p) h -> nb p h", p=P)
        o_t = o_d.rearrange("(nb p) h -> nb p h", p=P)

        for b in range(n_blocks):
            # spike_count for the whole block, via SWDGE (Pool-issued) so
            # the slot-reuse WAR against Pool's own accesses is free.
            tsc = io.tile([P, H], f32, tag="s")
            nc.gpsimd.dma_start(out=tsc[:], in_=s_t[b])
            tout = io.tile([P, H], f32, tag="out")

            for hf in range(2):
                sl = bass.ts(hf, HALF)
                txm = io.tile([P, 2, HALF], f32, tag="xm")
                nc.sync.dma_start(out=txm[:], in_=xm_t[b, hf])

                # m = x + mem   (DVE; only cross dep = the xm DMA)
                tm = work.tile([P, HALF], f32, tag="tm")
                nc.vector.tensor_tensor(tm[:], txm[:, 0, :], txm[:, 1, :], Alu.add)
                # q = m * (1/th), in place; alternate Pool/DVE to balance
                # (Pool work also steals a shared SBUF port from DVE).
                if hf == 0:
                    nc.gpsimd.tensor_tensor(tm[:], tm[:], R[:, sl], Alu.mult)
                else:
                    nc.vector.tensor_tensor(tm[:], tm[:], R[:, sl], Alu.mult)
                # rounding chain on ACT (own SBUF port, plenty of headroom):
                # ta = q + 0.5 ; ts2 = ta + C = C + ceil(q)
                ta = work2.tile([P, HALF], f32, tag="tab")
                nc.scalar.activation(ta[:], tm[:], Act.Identity, bias=bias_half[:])
                tb = work2.tile([P, HALF], f32, tag="tab")
                nc.scalar.activation(tb[:], ta[:], Act.Identity, bias=bias_C[:])
                # k_pos = relu(ts2 - (C+1)), then j_mem = relu(-ts2 + C); kp
                # first so the min's wait on jm's tick also covers kp.
                tkq = work2.tile([P, HALF], f32, tag="tkq")
                nc.scalar.activation(tkq[:], tb[:], Act.Relu, bias=bias_kp[:])
                tj = work2.tile([P, HALF], f32, tag="tj")
                nc.scalar.activation(
                    tj[:], tb[:], Act.Relu, bias=bias_jm[:], scale=-1.0
                )
                # s = spike_count / th   (Pool, in place)
                nc.gpsimd.tensor_tensor(tsc[:, sl], tsc[:, sl], R[:, sl], Alu.mult)
                # DVE pre-observes Pool's s tick, then the min needs only
                # the ACT wait.
                c0 = hf * HALF
                i_obs = nc.vector.tensor_copy(dve_dummy[:], tsc[:, c0 : c0 + 1])
                # kn = min(j_mem, s)
                i_min = nc.vector.tensor_tensor(tj[:], tj[:], tsc[:, sl], Alu.min)
                add_dep_helper(i_min.ins, i_obs.ins, sync=False, reason="obs<min")
                # d = k_pos - kn on the (otherwise idle) TensorEngine:
                # psum = I.T @ kp + (-I).T @ kn, exact for small integers.
                td = psum.tile([P, HALF], f32, tag="td")
                for c in range(HALF // NMM):
                    cs = bass.ts(c, NMM)
                    nc.tensor.matmul(
                        td[:, cs], eye[:, 0:P], tkq[:, cs], start=True, stop=False
                    )
                    nc.tensor.matmul(
                        td[:, cs], eye[:, P : 2 * P], tj[:, cs],
                        start=False, stop=True,
                    )
                # spike = d * th
                nc.vector.tensor_tensor(tout[:, sl], td[:], TH[:, sl], Alu.mult)

            nc.sync.dma_start(out=o_t[b], in_=tout[:])

    return nc


def kernel(**inputs: np.ndarray) -> np.ndarray:
    from concourse.bass_utils import run_bass_kernel_spmd

    x = np.ascontiguousarray(inputs["x"], dtype=np.float32).reshape(B * T, H)
    mem = np.ascontiguousarray(inputs["mem"], dtype=np.float32).reshape(B * T, H)
    sc = np.ascontiguousarray(inputs["spike_count"], dtype=np.float32).reshape(
        B * T, H
    )
    th = np.ascontiguousarray(inputs["threshold"], dtype=np.float32)
    xm = np.concatenate([x, mem], axis=1)  # [B*T, 2H]
    eye = np.concatenate(
        [np.eye(P, dtype=np.float32), -np.eye(P, dtype=np.float32)], axis=1
    )

    if "nc" not in _NC_CACHE:
        nc = build_nc()
        nc.finalize()
        _NC_CACHE["nc"] = nc
    nc = _NC_CACHE["nc"]

    r = ROWS_PER_CORE
    in_maps = [
        {
            "xm": xm[c * r : (c + 1) * r],
            "spike_count": sc[c * r : (c + 1) * r],
            "threshold": th,
            "eye": eye,
        }
        for c in range(N_CORES)
    ]
    res = run_bass_kernel_spmd(nc, in_maps, core_ids=list(range(N_CORES)))
    out = np.concatenate([res.results[c]["spike"] for c in range(N_CORES)], axis=0)
    return out.reshape(B, T, H).astype(np.float32, copy=False)
